# revision 44
# baseline (speedup 1.0000x reference)
"""Trainium2 Bass kernel for nn_DenoiseNet (langevin point-cloud denoiser).

Strategy (8 NeuronCores, SPMD, zero inter-core communication):
  - Shard over B(2) x 4 contiguous N-chunks of 4096 points with a 32-point
    halo (dependency cone grows 3 pts/step, 4 steps -> 12 needed). Global
    edge clipping handled exactly via per-core weight data (zeros on
    interior cores) so one program runs on all cores.
  - Feature-major fp16 layout [128 feat, cols]. Per step:
      U   = W0g.delta                       (PE, 3-contraction; ACT copy)
      h0  = relu(U[n+off_k] + Gk[n,k])      (DVE TT 2x + TS-max 4x)
      r1  = relu(Wb1.h0 + bb1)              (PE + ACT/DVE)
      h2  = h0 + r1                         (DVE/Pool TT)
      r2  = relu(Wb2.h2 + bb2)              (PE + ACT/DVE)
      h2 += r2                              (in-place TT, DVE/Pool)
      scatter: sum_k WoS.h2[n-off_k]        (PE, k-shifted access patterns)
      delta' = scatter + delta + s.bo.c     (I4 inject + ACT/DVE copy)
    Gk = G0[n] + A0e[n+off_k] precomputed once (constant across steps).
  - Computing h0 on the vector engines (instead of PE identity-injects)
    cuts PE work 40% vs the v1 kernel; 1024-col elementwise ops (PSUM ops
    span two banks) halve the per-instruction overhead; all elementwise
    work is greedily load-balanced across ACT/DVE/Pool with cost-model
    estimates (Pool/Q7 only sees SBUF fp16 tensor-tensor ops).
"""

import sys
import numpy as np

for _p in ("/opt/trn_rl_repo",):
    if _p not in sys.path:
        sys.path.insert(0, _p)

import concourse.bass as bass
import concourse.bacc as bacc
import concourse.tile as tile
from concourse import mybir
from concourse.bass_utils import run_bass_kernel_spmd

# ---- problem constants (hardcoded per harness contract) ----
B, N, D = 2, 16384, 3
F = 128
K = 4
OFF = [-2, -1, 0, 1]
STEPS, S0, DECAY = 4, 0.2, 0.95
CHUNK, HALO, GW = 4096, 32, 2
NP = CHUNK + 2 * HALO          # 4160 local points
NB = NP + 2 * GW               # 4228 cols with guards
R4 = K * NP                    # 16896 h2 columns
N_CORES = 8

f32 = mybir.dt.float32
f16 = mybir.dt.float16
AF = mybir.ActivationFunctionType
ALU = mybir.AluOpType

BW = 1024                      # EW block width (PSUM ops span 2 banks)
_CH = [(c * BW, min(BW, NP - c * BW)) for c in range((NP + BW - 1) // BW)]
_CHNB = [(c * BW, min(BW, NB - c * BW)) for c in range((NB + BW - 1) // BW)]
NCB = len(_CH)                 # 5


def build_program(reps=1, loop_n=0):
    nc = bacc.Bacc("TRN2", target_bir_lowering=False, debug=False)

    def inp(name, shape, dt):
        return nc.dram_tensor(name, list(shape), dt, kind="ExternalInput").ap()

    d_pclT = inp("pclT", (4, NB), f16)
    d_delta0 = inp("delta0", (4, NB), f16)
    d_Wf1 = inp("Wf1", (3, F), f16)
    d_bf1 = inp("bf1", (F, 1), f32)
    d_WfW = inp("WfW", (F, F), f16)
    d_bg = inp("bg", (F, 1), f32)
    d_W0g = inp("W0g", (3, F), f16)
    d_W0gn = inp("W0gn", (3, F), f16)
    d_Wb1 = inp("Wb1", (F, F), f16)
    d_Wb2 = inp("Wb2", (F, F), f16)
    d_bb1 = inp("bb1", (F, 1), f32)
    d_bb2 = inp("bb2", (F, 1), f32)
    d_WoS = inp("WoS", (F, 3 * STEPS), f16)
    d_I4 = inp("I4aug", (4, 3 * STEPS), f16)
    d_eL = inp("eL", (F, 3 * STEPS), f16)
    d_eLn = inp("eLn", (F, 3 * STEPS), f16)
    d_eR = inp("eR", (F, 3 * STEPS), f16)
    d_eRn = inp("eRn", (F, 3 * STEPS), f16)
    d_flagL = inp("flagL", (4, 1), f32)
    d_flagR = inp("flagR", (4, 1), f32)
    d_out = nc.dram_tensor("outT", [4, CHUNK], f16, kind="ExternalOutput").ap()

    from contextlib import ExitStack
    with tile.TileContext(nc) as tc, ExitStack() as ctx:
        cpool = ctx.enter_context(tc.tile_pool(name="const", bufs=1))
        prepool = ctx.enter_context(tc.tile_pool(name="pre", bufs=2))
        hppool = ctx.enter_context(tc.tile_pool(name="hpp", bufs=4))
        h0pool = ctx.enter_context(tc.tile_pool(name="h0p", bufs=8))
        r1pool = ctx.enter_context(tc.tile_pool(name="r1p", bufs=4))
        r2pool = ctx.enter_context(tc.tile_pool(name="r2p", bufs=4))
        tpool = ctx.enter_context(tc.tile_pool(name="tiny", bufs=4))
        psp = ctx.enter_context(tc.tile_pool(name="ps", bufs=3, space="PSUM"))
        pspp = ctx.enter_context(tc.tile_pool(name="psP", bufs=2, space="PSUM"))

        def load(dram, shape, dt, tag):
            t = cpool.tile(list(shape), dt, tag=tag)
            nc.sync.dma_start(t[:], dram[:])
            return t

        pclT = load(d_pclT, (4, NB), f16, "pclT")
        delta_a = load(d_delta0, (4, NB), f16, "delta_a")
        delta_b = load(d_delta0, (4, NB), f16, "delta_b")
        Wf1 = load(d_Wf1, (3, F), f16, "Wf1")
        bf1 = load(d_bf1, (F, 1), f32, "bf1")
        WfW = load(d_WfW, (F, F), f16, "WfW")
        bg = load(d_bg, (F, 1), f32, "bg")
        W0g = load(d_W0g, (3, F), f16, "W0g")
        W0gn = load(d_W0gn, (3, F), f16, "W0gn")
        Wb1 = load(d_Wb1, (F, F), f16, "Wb1")
        Wb2 = load(d_Wb2, (F, F), f16, "Wb2")
        bb1 = load(d_bb1, (F, 1), f32, "bb1")
        bb2 = load(d_bb2, (F, 1), f32, "bb2")
        WoS = load(d_WoS, (F, 3 * STEPS), f16, "WoS")
        I4 = load(d_I4, (4, 3 * STEPS), f16, "I4")
        eL = load(d_eL, (F, 3 * STEPS), f16, "eL")
        eLn = load(d_eLn, (F, 3 * STEPS), f16, "eLn")
        eR = load(d_eR, (F, 3 * STEPS), f16, "eR")
        eRn = load(d_eRn, (F, 3 * STEPS), f16, "eRn")
        flagL = load(d_flagL, (4, 1), f32, "flagL")
        flagR = load(d_flagR, (4, 1), f32, "flagR")

        A0e = cpool.tile([F, NB], f16, tag="A0e")
        G0 = cpool.tile([F, NP], f16, tag="G0")
        Gk = cpool.tile([F, R4], f16, tag="Gk")
        U_a = cpool.tile([F, NB], f16, tag="U_a")
        U_b = cpool.tile([F, NB], f16, tag="U_b")
        h2_a = cpool.tile([F, R4], f16, tag="h2_a")
        h2_b = cpool.tile([F, R4], f16, tag="h2_b")

        # ---- LP-constrained greedy engine balancer ----
        load_ns = {"ACT": 0.0, "DVE": 0.0, "POOL": 0.0}

        def pick(cands):
            def score(c):
                return max(load_ns[e] + cost for e, cost, _ in c)
            best = min(cands, key=score)
            for e, cost, fn in best:
                load_ns[e] += cost
                fn()

        def c_act(fd):
            return (fd + 212) * 0.833 + 16

        def c_dve_psum(fd):
            return (fd + 120) * 1.042 + 15

        def c_dve_tt16(fd):
            return (fd / 2 + 58) * 1.042 + 15

        def c_pool_tt(fd):
            return fd * 1.98 + 131

        def relu_psum(dst, src, fd, bias, eng=None):
            pick([
                [("ACT", c_act(fd), lambda: nc.scalar.activation(
                    dst, src, AF.Relu, bias=bias[:, :]))],
                [("DVE", c_dve_psum(fd), lambda: nc.vector.tensor_scalar(
                    dst, src, bias[:, :], 0.0, ALU.add, ALU.max))],
            ])

        def copy_psum(dst, src, fd, eng=None):
            pick([
                [("ACT", c_act(fd), lambda: nc.scalar.activation(
                    dst, src, AF.Copy))],
                [("DVE", c_dve_psum(fd), lambda: nc.vector.tensor_copy(
                    dst, src))],
            ])

        def tt16(dst, a, b, fd, eng=None):
            # adds: DVE (2x mode) or Pool; h0pre is forced DVE by the caller
            if eng == "DVE":
                load_ns["DVE"] += c_dve_tt16(fd)
                nc.vector.tensor_add(dst, a, b)
                return
            pick([
                [("DVE", c_dve_tt16(fd), lambda: nc.vector.tensor_add(dst, a, b))],
                [("POOL", c_pool_tt(fd), lambda: nc.gpsimd.tensor_add(dst, a, b))],
            ])

        def ts_relu16(dst, src, fd, eng=None):
            load_ns["DVE"] += (fd / 4 + 58) * 1.042 + 15
            nc.vector.tensor_scalar_max(dst, src, 0.0)

        def mm_halves(ps, lhsT, rhs_fn, fd, **kw):
            for h0c in range(0, fd, 512):
                hw = min(512, fd - h0c)
                nc.tensor.matmul(ps[:, h0c:h0c + hw], lhsT, rhs_fn(h0c, hw), **kw)

        # ---------------- preamble: A0e, G0, Gk (block-interleaved) ----------
        def pre_A0e(b):
            c0, fd = _CHNB[b]
            ps = psp.tile([F, BW], f32, tag="ps")
            mm_halves(ps, W0g[:, :], lambda h, w: pclT[0:3, c0 + h:c0 + h + w], fd,
                      start=True, stop=True)
            copy_psum(A0e[:, c0:c0 + fd], ps[:, :fd], fd, "ACT")

        def pre_G0(b):
            c0, fd = _CH[b]
            ps = psp.tile([F, BW], f32, tag="ps")
            mm_halves(ps, Wf1[:, :], lambda h, w: pclT[0:3, GW + c0 + h:GW + c0 + h + w],
                      fd, start=True, stop=True)
            hf = prepool.tile([F, BW], f16, tag="hf")
            nc.scalar.activation(hf[:, :fd], ps[:, :fd], AF.Relu, bias=bf1[:, :])
            ps2 = psp.tile([F, BW], f32, tag="ps")
            for h0c in range(0, fd, 512):
                hw = min(512, fd - h0c)
                nc.tensor.matmul(ps2[:, h0c:h0c + hw], WfW[:, :], hf[:, h0c:h0c + hw],
                                 start=True, stop=False)
                nc.tensor.matmul(ps2[:, h0c:h0c + hw], W0gn[:, :],
                                 pclT[0:3, GW + c0 + h0c:GW + c0 + h0c + hw],
                                 start=False, stop=True)
            nc.scalar.activation(G0[:, c0:c0 + fd], ps2[:, :fd], AF.Identity,
                                 bias=bg[:, :])

        def pre_Gk(b):
            c0, fd = _CH[b]
            for k in range(K):
                tt16(Gk[:, k * NP + c0:k * NP + c0 + fd], G0[:, c0:c0 + fd],
                     A0e[:, GW + OFF[k] + c0:GW + OFF[k] + c0 + fd], fd)

        for b in range(len(_CHNB) + 3):
            if b < len(_CHNB):
                pre_A0e(b)
            if 0 <= b - 1 < NCB:
                pre_G0(b - 1)
            if 0 <= b - 3 < NCB:
                pre_Gk(b - 3)

        # scatter reads up to 2 cols past each k-block edge; the cols just
        # outside the written range must not be fp16 garbage (NaN) on step 0
        for h2t in (h2_a, h2_b):
            nc.vector.memset(h2t[:, NP - 2:NP + 2], 0.0)
            nc.vector.memset(h2t[:, 2 * NP - 2:2 * NP + 2], 0.0)
            nc.vector.memset(h2t[:, 3 * NP - 2:3 * NP + 2], 0.0)
            nc.vector.memset(h2t[:, R4 - 2:R4], 0.0)
            nc.vector.memset(h2t[:, 0:2], 0.0)

        # ---------------- langevin steps ----------------
        lN = HALO + CHUNK - 1

        def emit_step(step, final, first_step=False):
            d_in = delta_a if step % 2 == 0 else delta_b
            d_out_t = delta_b if step % 2 == 0 else delta_a
            h2 = h2_a if step % 2 == 0 else h2_b
            U16 = U_a if step % 2 == 0 else U_b
            s3 = slice(3 * step, 3 * step + 3)

            h0s = {}

            def emit_U(cb):
                c0, fd = _CHNB[cb]
                ps = psp.tile([F, BW], f32, tag="ps")
                mm_halves(ps, W0g[:, :], lambda h, w: d_in[0:3, c0 + h:c0 + h + w],
                          fd, start=True, stop=True)
                copy_psum(U16[:, c0:c0 + fd], ps[:, :fd], fd, "ACT")

            def emit_A(cb):
                c0, fd = _CH[cb]
                for k in range(K):
                    hp = hppool.tile([F, BW], f16, tag="h0pre")
                    if first_step:
                        # delta0 == 0 so U == 0: h0pre = G0 + A0e[shift] = Gk,
                        # computed directly (no U pass, no Gk dependency)
                        tt16(hp[:, :fd], G0[:, c0:c0 + fd],
                             A0e[:, GW + OFF[k] + c0:GW + OFF[k] + c0 + fd],
                             fd, "DVE")
                    else:
                        tt16(hp[:, :fd],
                             U16[:, GW + OFF[k] + c0:GW + OFF[k] + c0 + fd],
                             Gk[:, k * NP + c0:k * NP + c0 + fd], fd, "DVE")
                    h0 = h0pool.tile([F, BW], f16, tag="h0")
                    ts_relu16(h0[:, :fd], hp[:, :fd], fd, "DVE")
                    h0s[(k, cb)] = h0

            def emit_B(cb):
                c0, fd = _CH[cb]
                r1s = {}
                for k in range(K):
                    h0 = h0s[(k, cb)]
                    ps = psp.tile([F, BW], f32, tag="ps")
                    mm_halves(ps, Wb1[:, :], lambda h, w: h0[:, h:h + w], fd,
                              start=True, stop=True)
                    r1 = r1pool.tile([F, BW], f16, tag="r1")
                    relu_psum(r1[:, :fd], ps[:, :fd], fd, bb1, "ACT")
                    r1s[k] = r1
                for k in range(K):
                    hcol = k * NP + c0
                    tt16(h2[:, hcol:hcol + fd], h0s[(k, cb)][:, :fd],
                         r1s[k][:, :fd], fd)

            def emit_C(cb):
                c0, fd = _CH[cb]
                r2s = {}
                for k in range(K):
                    hcol = k * NP + c0
                    ps = psp.tile([F, BW], f32, tag="ps")
                    mm_halves(ps, Wb2[:, :],
                              lambda h, w: h2[:, hcol + h:hcol + h + w], fd,
                              start=True, stop=True)
                    r2 = r2pool.tile([F, BW], f16, tag="r2")
                    relu_psum(r2[:, :fd], ps[:, :fd], fd, bb2)
                    r2s[k] = r2
                for k in range(K):
                    hcol = k * NP + c0
                    tt16(h2[:, hcol:hcol + fd], h2[:, hcol:hcol + fd],
                         r2s[k][:, :fd], fd)

            def emit_scatter(cb):
                c0, fd = _CH[cb]
                for h0c in range(0, fd, 512):
                    hw = min(512, fd - h0c)
                    sc0 = c0 + h0c
                    ps = pspp.tile([4, 512], f32, tag="psP")
                    for k in range(K):
                        st = k * NP + sc0 - OFF[k]
                        nc.tensor.matmul(ps[0:3, :hw], WoS[:, s3],
                                         h2[:, st:st + hw],
                                         start=(k == 0), stop=False)
                    if cb == 0 and h0c == 0:
                        pcol = ps[0:3, HALO:HALO + 1]
                        for col, w in ((HALO, eL), (HALO + 1, eL),
                                       (NP + HALO, eL), (3 * NP + HALO - 1, eLn)):
                            nc.tensor.matmul(pcol, w[:, s3], h2[:, col:col + 1],
                                             start=False, stop=False)
                    if cb == NCB - 1 and h0c + hw == fd:
                        pN = ps[0:3, lN - sc0:lN - sc0 + 1]
                        for col, w in ((3 * NP + lN, eR), (lN + 2, eRn),
                                       (NP + lN + 1, eRn)):
                            nc.tensor.matmul(pN, w[:, s3], h2[:, col:col + 1],
                                             start=False, stop=False)
                        nc.tensor.matmul(ps[0:3, lN - 1 - sc0:lN - sc0], eRn[:, s3],
                                         h2[:, lN + 1:lN + 2], start=False, stop=False)
                    nc.tensor.matmul(ps[0:3, :hw], I4[:, s3],
                                     d_in[0:4, GW + sc0:GW + sc0 + hw],
                                     start=False, stop=True)
                    copy_psum(d_out_t[0:3, GW + sc0:GW + sc0 + hw], ps[0:3, :hw], hw, "DVE")

            def mirror_fix(flag, src_l, dst_ls):
                for dst_l in dst_ls:
                    t = tpool.tile([4, 1], f16, tag="mir")
                    nc.vector.tensor_sub(t[0:3, :],
                                         d_out_t[0:3, GW + src_l:GW + src_l + 1],
                                         d_out_t[0:3, GW + dst_l:GW + dst_l + 1])
                    nc.vector.tensor_scalar_mul(t[0:3, :], t[0:3, :], flag[0:3, :])
                    nc.vector.tensor_add(d_out_t[0:3, GW + dst_l:GW + dst_l + 1],
                                         d_out_t[0:3, GW + dst_l:GW + dst_l + 1],
                                         t[0:3, :])

            for cb in range(NCB + 6):
                if cb < len(_CHNB) and not first_step:
                    emit_U(cb)
                if 0 <= cb - 2 < NCB:
                    emit_A(cb - 2)
                if 0 <= cb - 3 < NCB:
                    emit_B(cb - 3)
                if 0 <= cb - 4 < NCB:
                    emit_C(cb - 4)
                if 0 <= cb - 6 < NCB:
                    emit_scatter(cb - 6)
                    if not final and cb - 6 == 0:
                        mirror_fix(flagL, HALO, (HALO - 2, HALO - 1))
                    if not final and cb - 6 == NCB - 1:
                        mirror_fix(flagR, HALO + CHUNK - 1, (HALO + CHUNK,))

            if final:
                nc.sync.dma_start(d_out[:, :],
                                  d_out_t[0:4, GW + HALO:GW + HALO + CHUNK])

        def emit_rep(final_rep, first_rep=False):
            for step in range(STEPS):
                emit_step(step, (step == STEPS - 1) and final_rep,
                          first_step=(first_rep and step == 0))

        if loop_n:
            with tc.For_i(0, loop_n, 1):
                emit_rep(False)
            emit_rep(True)
        else:
            for rep in range(reps):
                emit_rep(rep == reps - 1, first_rep=(rep == 0))

    nc.compile()
    return nc


def host_prep(inputs):
    """Slice/transpose/pad inputs per core; build weight-variant constants."""
    hf = np.float16

    pcl = np.asarray(inputs["pcl_noisy"], np.float32)
    Wf1 = np.asarray(inputs["Wf1"], np.float32)
    bf1 = np.asarray(inputs["bf1"], np.float32)
    Wf2 = np.asarray(inputs["Wf2"], np.float32)
    bf2 = np.asarray(inputs["bf2"], np.float32)
    W0 = np.asarray(inputs["W0"], np.float32)
    b0 = np.asarray(inputs["b0"], np.float32)
    Wb = np.asarray(inputs["Wb"], np.float32)
    bb = np.asarray(inputs["bb"], np.float32)
    Wo = np.asarray(inputs["Wo"], np.float32)
    bo = np.asarray(inputs["bo"], np.float32)

    W0g = W0[:3]
    WfW = Wf2 @ W0[3:]
    bg = bf2 @ W0[3:] + b0
    offs = np.arange(-(K - 1) // 2, (K - 1) // 2 + 1)
    nbr = np.clip(np.arange(N)[:, None] + offs, 0, N - 1).reshape(-1)
    c_global = np.bincount(nbr, minlength=N).astype(np.float32)

    svals = [S0 * DECAY ** i for i in range(STEPS)]
    WoS = np.concatenate([s * Wo for s in svals], axis=1)          # [128, 12]
    I4 = np.zeros((4, 3 * STEPS), np.float32)
    for i, s in enumerate(svals):
        blk = np.eye(4, 3, dtype=np.float32)
        blk[3, 0:3] = s * bo
        I4[:, 3 * i:3 * i + 3] = blk

    WoSe = WoS.astype(hf)
    zeros_e = np.zeros((F, 3 * STEPS), hf)

    shared = {
        "Wf1": Wf1.astype(hf), "bf1": bf1.reshape(F, 1),
        "WfW": WfW.astype(hf), "bg": bg.reshape(F, 1),
        "W0g": W0g.astype(hf), "W0gn": (-W0g).astype(hf),
        "Wb1": Wb[0].astype(hf), "Wb2": Wb[1].astype(hf),
        "bb1": bb[0].reshape(F, 1), "bb2": bb[1].reshape(F, 1),
        "WoS": WoSe,
        "I4aug": I4.astype(hf),
    }
    in_maps = []
    for core in range(N_CORES):
        b, ch = core // 4, core % 4
        g0 = ch * CHUNK - HALO
        idx = np.clip(np.arange(g0 - GW, g0 + NP + GW), 0, N - 1)
        pclT = np.empty((4, NB), hf)
        pclT[0:3] = pcl[b, idx].T.astype(hf)
        pclT[3] = 0.0
        delta0 = np.zeros((4, NB), hf)
        delta0[3, GW:GW + NP] = c_global[np.clip(np.arange(g0, g0 + NP), 0, N - 1)]
        isL, isR = ch == 0, ch == 3
        m = dict(shared)
        m["pclT"] = pclT
        m["delta0"] = delta0
        m["eL"] = (WoSe if isL else zeros_e)
        m["eLn"] = ((-WoSe) if isL else zeros_e)
        m["eR"] = (WoSe if isR else zeros_e)
        m["eRn"] = ((-WoSe) if isR else zeros_e)
        m["flagL"] = np.full((4, 1), 1.0 if isL else 0.0, np.float32)
        m["flagR"] = np.full((4, 1), 1.0 if isR else 0.0, np.float32)
        in_maps.append(m)
    return in_maps


_CACHED = {}


def _get_program(reps=1):
    if reps not in _CACHED:
        _CACHED[reps] = build_program(reps)
    return _CACHED[reps]


def kernel(**inputs):
    nc = _get_program(1)
    in_maps = host_prep(inputs)
    res = run_bass_kernel_spmd(nc, in_maps, list(range(N_CORES)))
    pcl = np.asarray(inputs["pcl_noisy"], np.float32)
    out = np.empty((B, N, D), np.float32)
    for core in range(N_CORES):
        b, ch = core // 4, core % 4
        sl = slice(ch * CHUNK, (ch + 1) * CHUNK)
        out[b, sl] = pcl[b, sl] + res.results[core]["outT"][0:3].T.astype(np.float32)
    return out
